# Initial kernel scaffold
#
"""Trainium2 Bass kernel for nn_CausalMemory (reverse-causal decayed attention).

Math: out = ((qh @ xb.T) * W) @ xb @ VOB, where xb = x @ basis (rank-128),
qh = xb @ (Qc.T Kc), VOB = (Vc.T Oc) basis.T * out_scale, and
W[t,s] = decay^(s-t-1) for s>t else 0 (strictly-future attention).
decay^256 ~ 4e-6, so attention is windowed to the next J-1 chunks of 128.

Sharding: 8 cores = batch(4) x sequence-halves(2). Each core handles 2048
query tokens; its key/value range extends (J-1)*128 tokens past the query
range (zero-padded at the end of the sequence, which reproduces truncation
exactly).
"""

import numpy as np
import ml_dtypes

B, T, C, H = 4, 4096, 512, 128
TQ = 2048           # query tokens per core
CH = 128            # chunk
J = 2               # window chunks (own + J-1 ahead)
LOOK = (J - 1) * CH
TK = TQ + LOOK      # key tokens per core
NCH = TK // CH      # key chunks per core
NT = TQ // CH       # query tiles per core
BLK = 512           # token block
# block widths; block 0 split small so the first matmuls start ASAP
KBW = [128, 384] + [512] * ((TK - 512) // 512)
if sum(KBW) < TK:
    KBW.append(TK - sum(KBW))
NKB = len(KBW)
KBO = [sum(KBW[:b]) for b in range(NKB)]  # block token offsets

CB_W = 512 + 128 + 512 + 128      # basis | a_mat | vob | ident   (bf16)
CF_W = J * 128                    # wmask                         (f32)

_CACHE = {}

CFG = {
    "out_v": 2,        # out copies p < out_v -> vector, else scalar
    "warm": 0,
    "xtok_eng": "s",   # engine for xtok copy
    "qh_eng": "s",
    "xb_eng": "s",
    "rvg_eng": "v",
    "bufs_st": 8,
    "bufs_xt": 3,
    "bufs_rv": 3,
    "bufs_outb": 2,
    "ps_xq": 2, "ps_st": 2, "ps_rv": 2, "ps_out": 2,
}


def _build():
    import concourse.tile as tile
    from concourse import bacc, mybir

    bf16 = mybir.dt.bfloat16
    f32 = mybir.dt.float32

    nc = bacc.Bacc("TRN2", target_bir_lowering=False, debug=False, num_devices=8)

    xt_ext = nc.declare_dram_parameter("xt", [128, 4 * TK], bf16, isOutput=False)
    cb_ext = nc.declare_dram_parameter("cb", [128, CB_W], bf16, isOutput=False)
    cf_ext = nc.declare_dram_parameter("cf", [128, CF_W], f32, isOutput=False)
    out_ext = nc.declare_dram_parameter("out", [TQ, 512], bf16, isOutput=True)

    def _dma_l(dst, srcap):
        eng = nc.sync if CFG.get("dma_l", "sync") == "sync" else nc.gpsimd
        eng.dma_start(dst, srcap)

    def _dma_s(dst, srcap):
        eng = nc.sync if CFG.get("dma_s", "gp") == "sync" else nc.gpsimd
        eng.dma_start(dst, srcap)

    def _copy(eng, dst, srcap):
        if eng == "v":
            nc.vector.tensor_copy(dst, srcap)
        elif eng == "s":
            nc.scalar.copy(dst, srcap)
        else:
            nc.any.tensor_copy(dst, srcap)

    with tile.TileContext(nc) as tc:
        with (
            tc.tile_pool(name="consts", bufs=1) as cpool,
            tc.tile_pool(name="xt", bufs=NKB) as xtp,
            tc.tile_pool(name="big", bufs=1) as bigp,
            tc.tile_pool(name="st", bufs=NCH) as stp,
            tc.tile_pool(name="rv", bufs=4) as rvp,
            tc.tile_pool(name="outb", bufs=4) as outp,
            tc.tile_pool(name="ps_xq", bufs=CFG["ps_xq"], space="PSUM") as ps_xq,
            tc.tile_pool(name="ps_st", bufs=CFG["ps_st"], space="PSUM") as ps_stp,
            tc.tile_pool(name="ps_rv", bufs=CFG["ps_rv"], space="PSUM") as ps_rvp,
            tc.tile_pool(name="ps_out", bufs=CFG["ps_out"], space="PSUM") as ps_outp,
        ):
            cb = cpool.tile([128, CB_W], bf16)
            _dma_l(cb[:], cb_ext[:])
            cf = cpool.tile([128, CF_W], f32)
            _dma_l(cf[:], cf_ext[:])
            basis_s = cb[:, 0:512]
            a_s = cb[:, 512:640]
            vob_s = cb[:, 640:1152]
            id_s = cb[:, 1152:1280]
            wm_s = cf[:, 0:J * 128]

            if CFG.get("warm", 0):
                warm = cpool.tile([128, 128], bf16)
                nc.vector.memset(warm[:], 0)
                pwarm = ps_rvp.tile([128, 512], f32, tag="prv")
                for wi in range(CFG["warm"]):
                    nc.tensor.matmul(
                        pwarm[:, (wi % 4) * 128:(wi % 4) * 128 + 128],
                        warm[:], warm[:], start=True, stop=True)

            xb_big = bigp.tile([128, TK], bf16, tag="xb")
            xtok_big = bigp.tile([128, TK], bf16, tag="xtok")
            qh_big = bigp.tile([128, TQ], bf16, tag="qh")
            st_s = {}

            def block_stage(kb):
                w = KBW[kb]
                off = KBO[kb]
                xt3 = xtp.tile([128, 4, w], bf16, tag="xt")
                _dma_l(
                    xt3[:],
                    xt_ext[:, 4 * off:4 * off + 4 * w].rearrange(
                        "p (s t) -> p s t", s=4))
                pxb = ps_xq.tile([128, w], f32, tag="psxq")
                for sl in range(4):
                    nc.tensor.matmul(
                        pxb[:], basis_s[:, sl * 128:(sl + 1) * 128], xt3[:, sl, :],
                        start=(sl == 0), stop=(sl == 3))
                xb = xb_big[:, off:off + w]
                _copy(CFG["xb_eng"], xb, pxb[:])

                ptk = ps_xq.tile([128, w], bf16, tag="psxq")
                for ci in range(w // 128):
                    nc.tensor.transpose(
                        ptk[:, ci * 128:(ci + 1) * 128],
                        xb[:, ci * 128:(ci + 1) * 128], id_s)
                _copy(CFG["xtok_eng"], xtok_big[:, off:off + w], ptk[:])

                if off < TQ:
                    pqh = ps_xq.tile([128, w], f32, tag="psxq")
                    nc.tensor.matmul(pqh[:], a_s, xb, start=True, stop=True)
                    _copy(CFG["qh_eng"], qh_big[:, off:off + w], pqh[:])

            def scores_stage(c):
                n0 = max(0, c - (J - 1))
                n1 = min(NT - 1, c)
                L = n1 - n0 + 1
                pst = ps_stp.tile([128, J * 128], f32, tag="pst")
                nc.tensor.matmul(
                    pst[:, :L * 128],
                    xb_big[:, c * 128:(c + 1) * 128],
                    qh_big[:, n0 * 128:(n1 + 1) * 128],
                    start=True, stop=True)
                st = stp.tile([128, J * 128], bf16, tag="st")
                # wm slab k holds j=J-1-k; the needed j run (c-n0 .. c-n1) is a
                # contiguous slice of it
                w0 = (J - 1 - (c - n0)) * 128
                nc.vector.tensor_mul(st[:, :L * 128], pst[:, :L * 128],
                                     wm_s[:, w0:w0 + L * 128])
                st_s[c] = st

            def out_group(g):
                ob = outp.tile([128, 4, 512], bf16, tag="outb")
                prv = ps_rvp.tile([128, 4, 128], f32, tag="prv")
                for p in range(4):
                    i = g * 4 + p
                    for j in range(J):
                        c = i + j
                        pos = i - max(0, c - (J - 1))
                        nc.tensor.matmul(
                            prv[:, p, :],
                            xtok_big[:, c * 128:(c + 1) * 128],
                            st_s[c][:, pos * 128:(pos + 1) * 128],
                            start=(j == 0), stop=(j == J - 1))
                rvg = rvp.tile([128, 4, 128], bf16, tag="rv")
                _copy(CFG["rvg_eng"], rvg[:], prv[:])
                for p in range(4):
                    pout = ps_outp.tile([128, 512], f32, tag="pout")
                    nc.tensor.matmul(pout[:], rvg[:, p, :], vob_s,
                                     start=True, stop=True)
                    _copy("v" if p < CFG["out_v"] else "s", ob[:, p, :], pout[:])
                nc.gpsimd.dma_start(
                    out_ext[g * 512:(g + 1) * 512, :].rearrange(
                        "(s p) c -> p s c", p=128),
                    ob[:])

            emitted_c = 0
            emitted_g = 0
            for kb in range(NKB):
                block_stage(kb)
                chunks_done = (KBO[kb] + KBW[kb]) // 128
                while emitted_c < min(chunks_done, NCH):
                    scores_stage(emitted_c)
                    emitted_c += 1
                while emitted_g < NT // 4 and 4 * emitted_g + 4 + (J - 1) <= emitted_c:
                    out_group(emitted_g)
                    emitted_g += 1
            while emitted_c < NCH:
                scores_stage(emitted_c)
                emitted_c += 1
            while emitted_g < NT // 4:
                out_group(emitted_g)
                emitted_g += 1

    nc.compile()
    return nc


def _host_consts(basis, qc, kc, vc, oc, decay_logit, out_scale):
    bf = ml_dtypes.bfloat16
    d = 1.0 / (1.0 + np.exp(-np.float64(decay_logit)))
    basis64 = np.asarray(basis, np.float64)
    A = np.asarray(qc, np.float64).T @ np.asarray(kc, np.float64)
    VOB = (np.asarray(vc, np.float64).T @ np.asarray(oc, np.float64)) \
        @ basis64.T * np.float64(out_scale)
    # full decay mask, reversed slab order: slab k holds j = J-1-k.
    # value at [s, k*128+t] = d^(128j + s - t - 1) for j>=1;  j=0: tri.
    W = np.zeros((CH, J * CH), dtype=np.float64)
    s_idx = np.arange(CH)[:, None]
    t_idx = np.arange(CH)[None, :]
    for k in range(J):
        j = J - 1 - k
        if j == 0:
            W[:, k * CH:(k + 1) * CH] = np.where(
                s_idx > t_idx, d ** np.maximum(s_idx - t_idx - 1, 0), 0.0)
        else:
            W[:, k * CH:(k + 1) * CH] = d ** (CH * j + s_idx - t_idx - 1)

    cb = np.zeros((128, CB_W), dtype=bf)
    cb[:, 0:512] = basis64.astype(np.float32).reshape(4, 128, 128) \
        .transpose(1, 0, 2).reshape(128, 512).astype(bf)
    cb[:, 512:640] = A.astype(np.float32).astype(bf)
    cb[:, 640:1152] = VOB.astype(np.float32).astype(bf)
    cb[:, 1152:1280] = np.eye(128, dtype=np.float32).astype(bf)
    cf = W.astype(np.float32)
    return cb, cf


def make_in_maps(x, basis, q_coeffs, k_coeffs, v_coeffs, o_coeffs,
                 decay_logit, out_scale):
    bf = ml_dtypes.bfloat16
    cb, cf = _host_consts(basis, q_coeffs, k_coeffs, v_coeffs, o_coeffs,
                          decay_logit, out_scale)
    x = np.asarray(x, np.float32)
    in_maps = []
    for b in range(B):
        xbT = np.ascontiguousarray(x[b].T)  # [C, T]
        for h in range(2):
            q0 = h * TQ
            xs = np.zeros((C, TK), dtype=np.float32)
            avail = min(TK, T - q0)
            xs[:, :avail] = xbT[:, q0:q0 + avail]
            x4 = xs.reshape(4, 128, TK)
            xt_p = np.empty((128, 4 * TK), dtype=bf)
            for kb in range(NKB):
                off, w = KBO[kb], KBW[kb]
                xt_p[:, 4 * off:4 * off + 4 * w] = (
                    x4[:, :, off:off + w].transpose(1, 0, 2).reshape(128, 4 * w))
            in_maps.append({"xt": xt_p, "cb": cb, "cf": cf})
    return in_maps


def assemble_out(results):
    out = np.zeros((B, T, C), dtype=np.float32)
    for core in range(8):
        b, h = core // 2, core % 2
        out[b, h * TQ:(h + 1) * TQ, :] = np.asarray(
            results[core]["out"]).astype(np.float32)
    return out


def get_nc():
    if "nc" not in _CACHE:
        _CACHE["nc"] = _build()
    return _CACHE["nc"]


def kernel(x, basis, q_coeffs, k_coeffs, v_coeffs, o_coeffs,
           decay_logit, out_scale):
    from concourse.bass_utils import run_bass_kernel_spmd

    nc = get_nc()
    in_maps = make_in_maps(x, basis, q_coeffs, k_coeffs, v_coeffs, o_coeffs,
                           decay_logit, out_scale)
    res = run_bass_kernel_spmd(nc, in_maps, list(range(8)))
    return assemble_out(res.results)



# revision 1
# speedup vs baseline: 1.0682x; 1.0682x over previous
"""Trainium2 Bass kernel for nn_CausalMemory (reverse-causal decayed attention).

Math: out = ((qh @ xb.T) * W) @ xb @ VOB, where xb = x @ basis (rank-128),
qh = xb @ (Qc.T Kc), VOB = (Vc.T Oc) basis.T * out_scale, and
W[t,s] = decay^(s-t-1) for s>t else 0 (strictly-future attention).
decay^256 ~ 4e-6, so attention is windowed to the next J-1 chunks of 128.

Sharding: 8 cores = batch(4) x sequence-halves(2). Each core handles 2048
query tokens; its key/value range extends (J-1)*128 tokens past the query
range (zero-padded at the end of the sequence, which reproduces truncation
exactly).
"""

import numpy as np
import ml_dtypes

B, T, C, H = 4, 4096, 512, 128
TQ = 2048           # query tokens per core
CH = 128            # chunk
J = 2               # window chunks (own + J-1 ahead)
LOOK = (J - 1) * CH
TK = TQ + LOOK      # key tokens per core
NCH = TK // CH      # key chunks per core
NT = TQ // CH       # query tiles per core
BLK = 512           # token block
# block widths; block 0 split small so the first matmuls start ASAP
KBW = [128, 384] + [512] * ((TK - 512) // 512)
if sum(KBW) < TK:
    KBW.append(TK - sum(KBW))
NKB = len(KBW)
KBO = [sum(KBW[:b]) for b in range(NKB)]  # block token offsets

CB_W = 512 + 128 + 512 + 128      # basis | a_mat | vob | ident   (bf16)
CF_W = J * 128                    # wmask                         (f32)

_CACHE = {}

CFG = {
    "out_v": 2,        # out copies p < out_v -> vector, else scalar
    "warm": 0,
    "xtok_eng": "s",   # engine for xtok copy
    "qh_eng": "s",
    "xb_eng": "s",
    "rvg_eng": "v",
    "bufs_st": 8,
    "bufs_xt": 3,
    "bufs_rv": 3,
    "bufs_outb": 2,
    "ps_xq": 2, "ps_st": 2, "ps_rv": 2, "ps_out": 2,
}


def _build():
    import concourse.tile as tile
    from concourse import bacc, mybir

    bf16 = mybir.dt.bfloat16
    f32 = mybir.dt.float32

    nc = bacc.Bacc("TRN2", target_bir_lowering=False, debug=False, num_devices=8)

    xt_ext = nc.declare_dram_parameter("xt", [128, 4 * TK], bf16, isOutput=False)
    cb_ext = nc.declare_dram_parameter("cb", [128, CB_W], bf16, isOutput=False)
    cf_ext = nc.declare_dram_parameter("cf", [128, CF_W], f32, isOutput=False)
    out_ext = nc.declare_dram_parameter("out", [TQ, 512], bf16, isOutput=True)

    def _dma_l(dst, srcap):
        eng = nc.sync if CFG.get("dma_l", "sync") == "sync" else nc.gpsimd
        eng.dma_start(dst, srcap)

    def _dma_s(dst, srcap):
        eng = nc.sync if CFG.get("dma_s", "gp") == "sync" else nc.gpsimd
        eng.dma_start(dst, srcap)

    def _copy(eng, dst, srcap):
        if eng == "v":
            nc.vector.tensor_copy(dst, srcap)
        elif eng == "s":
            nc.scalar.copy(dst, srcap)
        else:
            nc.any.tensor_copy(dst, srcap)

    with tile.TileContext(nc) as tc:
        with (
            tc.tile_pool(name="consts", bufs=1) as cpool,
            tc.tile_pool(name="xt", bufs=NKB) as xtp,
            tc.tile_pool(name="big", bufs=1) as bigp,
            tc.tile_pool(name="st", bufs=NCH) as stp,
            tc.tile_pool(name="rv", bufs=4) as rvp,
            tc.tile_pool(name="outb", bufs=4) as outp,
            tc.tile_pool(name="ps_xq", bufs=CFG["ps_xq"], space="PSUM") as ps_xq,
            tc.tile_pool(name="ps_st", bufs=CFG["ps_st"], space="PSUM") as ps_stp,
            tc.tile_pool(name="ps_rv", bufs=CFG["ps_rv"], space="PSUM") as ps_rvp,
            tc.tile_pool(name="ps_out", bufs=CFG["ps_out"], space="PSUM") as ps_outp,
        ):
            cb = cpool.tile([128, CB_W], bf16)
            _dma_l(cb[:], cb_ext[:])
            cf = cpool.tile([128, CF_W], f32)
            _dma_l(cf[:], cf_ext[:])
            basis_s = cb[:, 0:512]
            a_s = cb[:, 512:640]
            vob_s = cb[:, 640:1152]
            id_s = cb[:, 1152:1280]
            wm_s = cf[:, 0:J * 128]

            if CFG.get("warm", 0):
                warm = cpool.tile([128, 128], bf16)
                nc.vector.memset(warm[:], 0)
                pwarm = ps_rvp.tile([128, 512], f32, tag="prv")
                for wi in range(CFG["warm"]):
                    nc.tensor.matmul(
                        pwarm[:, (wi % 4) * 128:(wi % 4) * 128 + 128],
                        warm[:], warm[:], start=True, stop=True)

            xb_big = bigp.tile([128, TK], bf16, tag="xb")
            xtok_big = bigp.tile([128, TK], bf16, tag="xtok")
            qh_big = bigp.tile([128, TQ], bf16, tag="qh")
            st_s = {}

            def block_stage(kb):
                w = KBW[kb]
                off = KBO[kb]
                xt3 = xtp.tile([128, 4, w], bf16, tag="xt")
                _dma_l(
                    xt3[:],
                    xt_ext[:, 4 * off:4 * off + 4 * w].rearrange(
                        "p (s t) -> p s t", s=4))
                pxb = ps_xq.tile([128, w], f32, tag="psxq")
                for sl in range(4):
                    nc.tensor.matmul(
                        pxb[:], basis_s[:, sl * 128:(sl + 1) * 128], xt3[:, sl, :],
                        start=(sl == 0), stop=(sl == 3))
                xb = xb_big[:, off:off + w]
                _copy(CFG["xb_eng"], xb, pxb[:])

                ptk = ps_xq.tile([128, w], bf16, tag="psxq")
                for ci in range(w // 128):
                    nc.tensor.transpose(
                        ptk[:, ci * 128:(ci + 1) * 128],
                        xb[:, ci * 128:(ci + 1) * 128], id_s)
                _copy(CFG["xtok_eng"], xtok_big[:, off:off + w], ptk[:])

                if off < TQ:
                    pqh = ps_xq.tile([128, w], f32, tag="psxq")
                    nc.tensor.matmul(pqh[:], a_s, xb, start=True, stop=True)
                    _copy(CFG["qh_eng"], qh_big[:, off:off + w], pqh[:])

            def scores_stage(c):
                n0 = max(0, c - (J - 1))
                n1 = min(NT - 1, c)
                L = n1 - n0 + 1
                pst = ps_stp.tile([128, J * 128], f32, tag="pst")
                nc.tensor.matmul(
                    pst[:, :L * 128],
                    xb_big[:, c * 128:(c + 1) * 128],
                    qh_big[:, n0 * 128:(n1 + 1) * 128],
                    start=True, stop=True)
                st = stp.tile([128, J * 128], bf16, tag="st")
                # wm slab k holds j=J-1-k; the needed j run (c-n0 .. c-n1) is a
                # contiguous slice of it
                w0 = (J - 1 - (c - n0)) * 128
                nc.vector.tensor_mul(st[:, :L * 128], pst[:, :L * 128],
                                     wm_s[:, w0:w0 + L * 128])
                st_s[c] = st

            def out_group(g):
                ob = outp.tile([128, 4, 512], bf16, tag="outb")
                prv = ps_rvp.tile([128, 4, 128], f32, tag="prv")
                for p in range(4):
                    i = g * 4 + p
                    for j in range(J):
                        c = i + j
                        pos = i - max(0, c - (J - 1))
                        nc.tensor.matmul(
                            prv[:, p, :],
                            xtok_big[:, c * 128:(c + 1) * 128],
                            st_s[c][:, pos * 128:(pos + 1) * 128],
                            start=(j == 0), stop=(j == J - 1))
                rvg = rvp.tile([128, 4, 128], bf16, tag="rv")
                _copy(CFG["rvg_eng"], rvg[:], prv[:])
                for p in range(4):
                    pout = ps_outp.tile([128, 512], f32, tag="pout")
                    nc.tensor.matmul(pout[:], rvg[:, p, :], vob_s,
                                     start=True, stop=True)
                    _copy("v" if p < CFG["out_v"] else "s", ob[:, p, :], pout[:])
                nc.gpsimd.dma_start(
                    out_ext[g * 512:(g + 1) * 512, :].rearrange(
                        "(s p) c -> p s c", p=128),
                    ob[:])

            emitted_c = 0
            emitted_g = 0
            for kb in range(NKB):
                block_stage(kb)
                chunks_done = (KBO[kb] + KBW[kb]) // 128
                while emitted_c < min(chunks_done, NCH):
                    scores_stage(emitted_c)
                    emitted_c += 1
                while emitted_g < NT // 4 and 4 * emitted_g + 4 + (J - 1) <= emitted_c:
                    out_group(emitted_g)
                    emitted_g += 1
            while emitted_c < NCH:
                scores_stage(emitted_c)
                emitted_c += 1
            while emitted_g < NT // 4:
                out_group(emitted_g)
                emitted_g += 1

    nc.compile()
    return nc


def _host_consts(basis, qc, kc, vc, oc, decay_logit, out_scale):
    bf = ml_dtypes.bfloat16
    d = 1.0 / (1.0 + np.exp(-np.float64(decay_logit)))
    basis64 = np.asarray(basis, np.float64)
    A = np.asarray(qc, np.float64).T @ np.asarray(kc, np.float64)
    VOB = (np.asarray(vc, np.float64).T @ np.asarray(oc, np.float64)) \
        @ basis64.T * np.float64(out_scale)
    # full decay mask, reversed slab order: slab k holds j = J-1-k.
    # value at [s, k*128+t] = d^(128j + s - t - 1) for j>=1;  j=0: tri.
    W = np.zeros((CH, J * CH), dtype=np.float64)
    s_idx = np.arange(CH)[:, None]
    t_idx = np.arange(CH)[None, :]
    for k in range(J):
        j = J - 1 - k
        if j == 0:
            W[:, k * CH:(k + 1) * CH] = np.where(
                s_idx > t_idx, d ** np.maximum(s_idx - t_idx - 1, 0), 0.0)
        else:
            W[:, k * CH:(k + 1) * CH] = d ** (CH * j + s_idx - t_idx - 1)

    cb = np.zeros((128, CB_W), dtype=bf)
    cb[:, 0:512] = basis64.astype(np.float32).reshape(4, 128, 128) \
        .transpose(1, 0, 2).reshape(128, 512).astype(bf)
    cb[:, 512:640] = A.astype(np.float32).astype(bf)
    cb[:, 640:1152] = VOB.astype(np.float32).astype(bf)
    cb[:, 1152:1280] = np.eye(128, dtype=np.float32).astype(bf)
    cf = W.astype(np.float32)
    return cb, cf


def make_in_maps(x, basis, q_coeffs, k_coeffs, v_coeffs, o_coeffs,
                 decay_logit, out_scale):
    bf = ml_dtypes.bfloat16
    cb, cf = _host_consts(basis, q_coeffs, k_coeffs, v_coeffs, o_coeffs,
                          decay_logit, out_scale)
    x = np.asarray(x, np.float32)
    in_maps = []
    for b in range(B):
        xbT = np.ascontiguousarray(x[b].T)  # [C, T]
        for h in range(2):
            q0 = h * TQ
            xs = np.zeros((C, TK), dtype=np.float32)
            avail = min(TK, T - q0)
            xs[:, :avail] = xbT[:, q0:q0 + avail]
            x4 = xs.reshape(4, 128, TK)
            xt_p = np.empty((128, 4 * TK), dtype=bf)
            for kb in range(NKB):
                off, w = KBO[kb], KBW[kb]
                xt_p[:, 4 * off:4 * off + 4 * w] = (
                    x4[:, :, off:off + w].transpose(1, 0, 2).reshape(128, 4 * w))
            in_maps.append({"xt": xt_p, "cb": cb, "cf": cf})
    return in_maps


def assemble_out(results):
    out = np.zeros((B, T, C), dtype=np.float32)
    for core in range(8):
        b, h = core // 2, core % 2
        out[b, h * TQ:(h + 1) * TQ, :] = np.asarray(
            results[core]["out"]).astype(np.float32)
    return out


def get_nc():
    if "nc" not in _CACHE:
        _CACHE["nc"] = _build()
    return _CACHE["nc"]


def kernel(x, basis, q_coeffs, k_coeffs, v_coeffs, o_coeffs,
           decay_logit, out_scale):
    from concourse.bass_utils import run_bass_kernel_spmd

    nc = get_nc()
    in_maps = make_in_maps(x, basis, q_coeffs, k_coeffs, v_coeffs, o_coeffs,
                           decay_logit, out_scale)
    res = run_bass_kernel_spmd(nc, in_maps, list(range(8)))
    return assemble_out(res.results)



# revision 6
# speedup vs baseline: 1.0955x; 1.0256x over previous
"""Trainium2 Bass kernel for nn_CausalMemory (reverse-causal decayed attention).

Math: out = ((qh @ xb.T) * W) @ xb @ VOB, where xb = x @ basis (rank-128),
qh = xb @ (Qc.T Kc), VOB = (Vc.T Oc) basis.T * out_scale, and
W[t,s] = decay^(s-t-1) for s>t else 0 (strictly-future attention).
decay^128 ~ 2e-3, so attention is windowed to the next chunk of 128.

Sharding: 8 cores = batch(4) x sequence-halves(2). Each core handles 2048
query tokens; its key range extends 128 tokens past the query range
(zero-padded at the end of the sequence = exact truncation).

v2: warm-up matmuls beat the HAM cold clock; one merged const DMA; flat
input block DMAs; paired score chunks with one fused mask-multiply each;
evacuation split across scalar+vector; partition-major output layout so the
output DMA is fully contiguous, issued via sync HWDGE.
"""

import numpy as np
import ml_dtypes

B, T, C = 4, 4096, 512
TQ = 2048           # query tokens per core
CH = 128            # chunk
TK = TQ + CH        # key tokens per core (one chunk lookahead)
NCH = TK // CH      # key chunks per core (17)
NT = TQ // CH       # query chunks per core (16)

# DMA blocks (token widths); compute sub-blocks are <=512 within each
DBW = [128, 512, 768, 768]
assert sum(DBW) == TK
DBO = [sum(DBW[:b]) for b in range(len(DBW))]

# consts layout (bf16): basis | a_mat | vob | ident | wmask
CB_W = 512 + 128 + 512 + 128 + 512

_CACHE = {}

CFG = {
    "warm": 10,          # warm-up matmuls (N=512 each)
    "out_sc": 5,         # out evac pairs on scalar (of 8); rest vector
    "qh_eng": "v",       # qh evac engine
    "rv_eng": "s",       # rv evac engine
    "xb_eng": "s",
}


def _build():
    import concourse.tile as tile
    from concourse import bacc, mybir

    bf16 = mybir.dt.bfloat16
    f32 = mybir.dt.float32

    nc = bacc.Bacc("TRN2", target_bir_lowering=False, debug=False, num_devices=8)

    xt_ext = nc.declare_dram_parameter("xt", [128, 4 * TK], bf16, isOutput=False)
    cb_ext = nc.declare_dram_parameter("cb", [128, CB_W], bf16, isOutput=False)
    out_ext = nc.declare_dram_parameter("out", [128, NT * 512], bf16, isOutput=True)

    def _copy(eng, dst, srcap):
        if eng == "v":
            nc.vector.tensor_copy(dst, srcap)
        else:
            nc.scalar.copy(dst, srcap)

    with tile.TileContext(nc) as tc:
        with (
            tc.tile_pool(name="consts", bufs=1) as cpool,
            tc.tile_pool(name="xt", bufs=3) as xtp,
            tc.tile_pool(name="big", bufs=1) as bigp,
            tc.tile_pool(name="st", bufs=5) as stp,
            tc.tile_pool(name="rv", bufs=3) as rvp,
            tc.tile_pool(name="outb", bufs=2) as outp,
            tc.tile_pool(name="ps_a", bufs=2, space="PSUM") as ps_a,
            tc.tile_pool(name="ps_st", bufs=2, space="PSUM") as ps_st,
            tc.tile_pool(name="ps_rv", bufs=2, space="PSUM") as ps_rv,
            tc.tile_pool(name="ps_out", bufs=1, space="PSUM") as ps_out,
        ):
            cb = cpool.tile([128, CB_W], bf16)
            nc.sync.dma_start(cb[:], cb_ext[:])
            basis_s = cb[:, 0:512]
            a_s = cb[:, 512:640]
            vob_s = cb[:, 640:1152]
            id_s = cb[:, 1152:1280]
            wm_s = cb[:, 1280:1792]   # [X | D | X | D] pattern, 4x128

            # warm-up: keep PE busy from t0 so HAM unthrottles before real work
            if CFG["warm"]:
                warm = cpool.tile([128, 128], bf16)
                warm5 = cpool.tile([128, 512], bf16)
                nc.vector.memset(warm[:], 0)
                nc.vector.memset(warm5[:], 0)
                for wi in range(CFG["warm"]):
                    pw = ps_out.tile([128, 1024], f32, tag="pout")
                    nc.tensor.matmul(pw[:, (wi % 2) * 512:(wi % 2) * 512 + 512],
                                     warm[:], warm5[:], start=True, stop=True)

            # input block DMAs (flat, contiguous)
            xts = []
            for kb, w in enumerate(DBW):
                off = DBO[kb]
                xt = xtp.tile([128, 4, w], bf16, tag="xt")
                nc.sync.dma_start(
                    xt[:], xt_ext[:, 4 * off:4 * (off + w)].rearrange(
                        "p (s t) -> p s t", s=4))
                xts.append(xt)

            xb_big = bigp.tile([128, TK], bf16, tag="xb")
            xtok_big = bigp.tile([128, TK], bf16, tag="xtok")
            qh_big = bigp.tile([128, TQ], bf16, tag="qh")
            st_s = {}

            def sub_blocks():
                for kb, bw in enumerate(DBW):
                    o = DBO[kb]
                    while bw > 0:
                        w = min(512, bw)
                        yield kb, o, w
                        o += w
                        bw -= w

            def block_stage(kb, off, w):
                xt = xts[kb]
                lo = off - DBO[kb]
                pxb = ps_a.tile([128, w], f32, tag="psa")
                for sl in range(4):
                    nc.tensor.matmul(
                        pxb[:], basis_s[:, sl * 128:(sl + 1) * 128],
                        xt[:, sl, lo:lo + w],
                        start=(sl == 0), stop=(sl == 3))
                xb = xb_big[:, off:off + w]
                _copy(CFG["xb_eng"], xb, pxb[:])

                ptk = ps_a.tile([128, w], bf16, tag="psa")
                for ci in range(w // 128):
                    nc.tensor.transpose(
                        ptk[:, ci * 128:(ci + 1) * 128],
                        xb[:, ci * 128:(ci + 1) * 128], id_s)
                nc.vector.tensor_copy(xtok_big[:, off:off + w], ptk[:])

                if off < TQ:
                    qw = min(w, TQ - off)
                    pqh = ps_a.tile([128, qw], f32, tag="psa")
                    nc.tensor.matmul(pqh[:], a_s, xb[:, 0:qw],
                                     start=True, stop=True)
                    _copy(CFG["qh_eng"], qh_big[:, off:off + qw], pqh[:])

            def scores_pair(cg):
                # key chunks c0=2cg, c0+1; per chunk 256 cols: [X_c | D_c]
                # X_c multiplies queries of chunk c-1, D_c queries of chunk c
                c0 = 2 * cg
                c1 = min(NCH, c0 + 2)
                pst = ps_st.tile([128, 512], f32, tag="pst")
                st = stp.tile([128, 512], bf16, tag="st")
                for c in range(c0, c1):
                    base = (c - c0) * 256
                    if c == 0:
                        nc.tensor.matmul(pst[:, 128:256],
                                         xb_big[:, 0:128], qh_big[:, 0:128],
                                         start=True, stop=True)
                    elif c == NCH - 1:
                        nc.tensor.matmul(pst[:, base:base + 128],
                                         xb_big[:, c * 128:(c + 1) * 128],
                                         qh_big[:, (c - 1) * 128:c * 128],
                                         start=True, stop=True)
                    else:
                        nc.tensor.matmul(pst[:, base:base + 256],
                                         xb_big[:, c * 128:(c + 1) * 128],
                                         qh_big[:, (c - 1) * 128:(c + 1) * 128],
                                         start=True, stop=True)
                    st_s[c] = st
                lo = 128 if c0 == 0 else 0
                hi = 128 if c1 == NCH and c1 - c0 == 1 else (c1 - c0) * 256
                nc.vector.tensor_mul(st[:, lo:hi], pst[:, lo:hi],
                                     wm_s[:, lo:hi])

            def out_group(g):
                # query chunks 4g..4g+3
                prv = ps_rv.tile([128, 4, 128], f32, tag="prv")
                for p in range(4):
                    i = g * 4 + p
                    nc.tensor.matmul(
                        prv[:, p, :],
                        xtok_big[:, i * 128:(i + 1) * 128],
                        st_s[i][:, (i % 2) * 256 + 128:(i % 2) * 256 + 256],
                        start=True, stop=False)
                    j = i + 1
                    nc.tensor.matmul(
                        prv[:, p, :],
                        xtok_big[:, j * 128:(j + 1) * 128],
                        st_s[j][:, (j % 2) * 256:(j % 2) * 256 + 128],
                        start=False, stop=True)
                rvg = rvp.tile([128, 4, 128], bf16, tag="rv")
                _copy(CFG["rv_eng"], rvg[:], prv[:])
                ob = outp.tile([128, 2048], bf16, tag="outb")
                for pp in range(2):
                    pout = ps_out.tile([128, 1024], f32, tag="pout")
                    for q in range(2):
                        nc.tensor.matmul(pout[:, q * 512:(q + 1) * 512],
                                         rvg[:, pp * 2 + q, :],
                                         vob_s, start=True, stop=True)
                    eng = "s" if (g * 2 + pp) < CFG["out_sc"] else "v"
                    _copy(eng, ob[:, pp * 1024:(pp + 1) * 1024], pout[:])
                nc.sync.dma_start(
                    out_ext[:, g * 2048:(g + 1) * 2048], ob[:])

            emitted_cg = 0
            emitted_g = 0
            n_cg = (NCH + 1) // 2  # 9
            for kb, off, w in sub_blocks():
                block_stage(kb, off, w)
                chunks_done = (off + w) // 128
                while emitted_cg < n_cg and \
                        min(NCH, (emitted_cg + 1) * 2) <= chunks_done:
                    scores_pair(emitted_cg)
                    emitted_cg += 1
                while emitted_g < NT // 4 and \
                        4 * emitted_g + 5 <= emitted_cg * 2:
                    out_group(emitted_g)
                    emitted_g += 1
            while emitted_cg < n_cg:
                scores_pair(emitted_cg)
                emitted_cg += 1
            while emitted_g < NT // 4:
                out_group(emitted_g)
                emitted_g += 1

    nc.compile()
    return nc


def _host_consts(basis, qc, kc, vc, oc, decay_logit, out_scale):
    bf = ml_dtypes.bfloat16
    d = 1.0 / (1.0 + np.exp(-np.float64(decay_logit)))
    basis64 = np.asarray(basis, np.float64)
    A = np.asarray(qc, np.float64).T @ np.asarray(kc, np.float64)
    VOB = (np.asarray(vc, np.float64).T @ np.asarray(oc, np.float64)) \
        @ basis64.T * np.float64(out_scale)
    # wm [128, 512] = [X | D | X | D]
    s_idx = np.arange(CH)[:, None]
    t_idx = np.arange(CH)[None, :]
    X = d ** (CH + s_idx - t_idx - 1)
    D = np.where(s_idx > t_idx, d ** np.maximum(s_idx - t_idx - 1, 0), 0.0)
    wm = np.concatenate([X, D, X, D], axis=1)

    cb = np.zeros((128, CB_W), dtype=bf)
    cb[:, 0:512] = basis64.astype(np.float32).reshape(4, 128, 128) \
        .transpose(1, 0, 2).reshape(128, 512).astype(bf)
    cb[:, 512:640] = A.astype(np.float32).astype(bf)
    cb[:, 640:1152] = VOB.astype(np.float32).astype(bf)
    cb[:, 1152:1280] = np.eye(128, dtype=np.float32).astype(bf)
    cb[:, 1280:1792] = wm.astype(np.float32).astype(bf)
    return cb


def make_in_maps(x, basis, q_coeffs, k_coeffs, v_coeffs, o_coeffs,
                 decay_logit, out_scale):
    bf = ml_dtypes.bfloat16
    cb = _host_consts(basis, q_coeffs, k_coeffs, v_coeffs, o_coeffs,
                      decay_logit, out_scale)
    x = np.asarray(x, np.float32)
    in_maps = []
    for b in range(B):
        xbT = np.ascontiguousarray(x[b].T)  # [C, T]
        for h in range(2):
            q0 = h * TQ
            xs = np.zeros((C, TK), dtype=np.float32)
            avail = min(TK, T - q0)
            xs[:, :avail] = xbT[:, q0:q0 + avail]
            x4 = xs.reshape(4, 128, TK)
            xt_p = np.empty((128, 4 * TK), dtype=bf)
            for kb, w in enumerate(DBW):
                off = DBO[kb]
                for s in range(4):
                    xt_p[:, 4 * off + s * w:4 * off + (s + 1) * w] = \
                        x4[s, :, off:off + w]
            in_maps.append({"xt": xt_p, "cb": cb})
    return in_maps


def assemble_out(results):
    out = np.zeros((B, T, C), dtype=np.float32)
    for core in range(8):
        b, h = core // 2, core % 2
        r = np.asarray(results[core]["out"]).astype(np.float32)
        out[b, h * TQ:(h + 1) * TQ, :] = \
            r.reshape(128, NT, 512).transpose(1, 0, 2).reshape(TQ, 512)
    return out


def get_nc():
    if "nc" not in _CACHE:
        _CACHE["nc"] = _build()
    return _CACHE["nc"]


def kernel(x, basis, q_coeffs, k_coeffs, v_coeffs, o_coeffs,
           decay_logit, out_scale):
    from concourse.bass_utils import run_bass_kernel_spmd

    nc = get_nc()
    in_maps = make_in_maps(x, basis, q_coeffs, k_coeffs, v_coeffs, o_coeffs,
                           decay_logit, out_scale)
    res = run_bass_kernel_spmd(nc, in_maps, list(range(8)))
    return assemble_out(res.results)
